# revision 4
# baseline (speedup 1.0000x reference)
"""Trainium2 Bass kernel for nn_KernelBlock_7387343749286 (sparse_attention).

Computes, for features [B=8, T=2048, C=128], const [1], scale [T]:
    gram[b,t,s] = <features[b,t,:], features[b,s,:]>
    K = (gram + const) + exp(-(sq_t + sq_s - 2*gram) / (2*scale_s^2)) + eps*I

Sharding: batch b across the 8 NeuronCores (data parallel), one 2048x2048
output per core. Within a core the T x T Gram matrix is tiled into
[128 x 1024] blocks flash-attention style.

Per-core device algorithm (uniform scale sigma, c = 1/(2*sigma^2)):
  X^T via PE transposes; Xtc = c*X^T.
  bank A (PSUM) = c*gram + (-(c/2)(sq_s - C0)) via one f32r matmul plus a
      K=1 rank-1 accumulate; ScalarE computes E = exp(A + bias_t) with
      bias_t = -(c/2)sq_t - (c/2)C0 (per-partition), all exactly
      -dist/(2 sigma^2).
  bank B (PSUM) = gram (+ eps*I on diagonal blocks via a bf16 identity
      accumulate).
  VectorE fuses the rest in one pass: out = (B + const) + E.
"""

import numpy as np

B, T, C = 8, 2048, 128
EPSILON = 1e-5
P = 128            # partitions
NB = T // P        # 16 row blocks
HALF = 1024        # column tile width (2 PSUM banks)
NH = T // HALF     # 2 column halves
C0 = float(C)      # centering constant for sq values (E[sq] = C)

_CACHE = {}


def _build(c: float, const_val: float):
    import concourse.bass as bass
    import concourse.mybir as mybir
    from concourse import bacc
    from concourse.tile import TileContext
    from concourse.masks import make_identity

    f32 = mybir.dt.float32
    f32r = mybir.dt.float32r
    bf16 = mybir.dt.bfloat16
    Alu = mybir.AluOpType
    Act = mybir.ActivationFunctionType

    nc = bacc.Bacc("TRN2", target_bir_lowering=False, debug=False)
    x = nc.dram_tensor("x", (T, C), f32, kind="ExternalInput")
    out = nc.dram_tensor("out", (T, T), f32, kind="ExternalOutput")
    x_ap = x.ap()
    out_ap = out.ap()

    with TileContext(nc) as tc:
        with (
            tc.tile_pool(name="const_pool", bufs=1) as cpool,
            tc.tile_pool(name="work_pool", bufs=1) as wpool,
        ):
            # ---------------- prologue ----------------
            ident = cpool.tile([P, P], f32)
            make_identity(nc, ident)
            epseye = cpool.tile([P, P], bf16)
            nc.vector.tensor_scalar_mul(epseye[:], ident[:], EPSILON)
            ident_bf = cpool.tile([P, P], bf16)
            nc.vector.tensor_copy(ident_bf[:], ident[:])

            ones_f32 = cpool.tile([1, P], f32)
            nc.vector.memset(ones_f32[:], 1.0)
            ones_r1 = cpool.tile([1, P], f32r)
            nc.vector.tensor_copy(ones_r1[:], ones_f32[:])
            negc_ones = cpool.tile([P, 1], f32)
            nc.vector.memset(negc_ones[:], -0.5 * c)

            # natural-layout X: partition = t within block, free = (block, c)
            xnat = wpool.tile([P, T], f32)
            x_blocked = x_ap.rearrange("(mb p) c -> p mb c", p=P)
            for mb in range(NB):
                nc.sync.dma_start(
                    xnat[:, mb * C:(mb + 1) * C], x_blocked[:, mb, :]
                )

            xt = cpool.tile([P, T], f32r)   # X^T: partition = c, free = t
            with tc.tile_pool(name="tp_psum", bufs=4, space="PSUM") as tpp:
                for mb in range(NB):
                    pt = tpp.tile([P, P], f32)
                    nc.tensor.transpose(
                        pt[:], xnat[:, mb * P:(mb + 1) * P], ident[:]
                    )
                    nc.scalar.copy(xt[:, mb * P:(mb + 1) * P], pt[:])

            # squared-feature sums
            xtsq = wpool.tile([P, T], f32)
            nc.vector.tensor_tensor(xtsq[:], xt[:], xt[:], Alu.mult)
            xnsq = wpool.tile([P, T], f32)
            nc.vector.tensor_tensor(xnsq[:], xnat[:], xnat[:], Alu.mult)

            # sqcol[t-in-block, mb] -> ACT bias:  -(c/2)*sq_t - (c/2)*C0
            sqcol_raw = cpool.tile([P, NB], f32)
            nc.vector.tensor_reduce(
                sqcol_raw[:],
                xnsq[:].rearrange("p (mb c) -> p mb c", mb=NB),
                mybir.AxisListType.X,
                Alu.add,
            )
            sqcol = cpool.tile([P, NB], f32)
            nc.vector.tensor_scalar(
                sqcol[:], sqcol_raw[:], -0.5 * c, -0.5 * c * C0, Alu.mult, Alu.add
            )

            # sqrow[s] (centered):  -(c/2)*(sq_s - C0), fp32 matmul for accuracy
            sqrow = cpool.tile([1, T], f32r)
            with tc.tile_pool(name="sr_psum", bufs=4, space="PSUM") as srp:
                for j in range(T // 512):
                    pr = srp.tile([1, 512], f32)
                    nc.tensor.matmul(
                        pr[:], negc_ones[:], xtsq[:, j * 512:(j + 1) * 512],
                        start=True, stop=True,
                    )
                    # center: add +(c/2)*C0
                    nc.scalar.activation(
                        sqrow[0:1, j * 512:(j + 1) * 512], pr[:],
                        Act.Copy, bias=0.5 * c * C0, scale=1.0,
                    )

            # scaled features for bank A
            xtc = cpool.tile([P, T], f32r)
            nc.vector.tensor_scalar_mul(xtc[:], xt[:], c)

            # ---------------- main loop ----------------
            with (
                tc.tile_pool(name="pa_psum", bufs=2, space="PSUM") as pap,
                tc.tile_pool(name="pb_psum", bufs=2, space="PSUM") as pbp,
                tc.tile_pool(name="e_pool", bufs=3) as epool,
                tc.tile_pool(name="o_pool", bufs=3) as opool,
            ):
                for mb in range(NB):
                    mtile = xt[:, mb * P:(mb + 1) * P]
                    for h in range(NH):
                        pa = pap.tile([P, HALF], f32)
                        pb = pbp.tile([P, HALF], f32)
                        for j in range(HALF // 512):
                            lo = h * HALF + j * 512          # global col offset
                            sl = slice(j * 512, (j + 1) * 512)  # psum cols
                            gsl = slice(lo, lo + 512)        # xt cols
                            # bank A: c*gram + rank-1 column term
                            nc.tensor.matmul(
                                pa[:, sl], mtile, xtc[:, gsl],
                                start=True, stop=False,
                            )
                            nc.tensor.matmul(
                                pa[:, sl], ones_r1[:], sqrow[0:1, gsl],
                                start=False, stop=True,
                            )
                            # bank B: gram (+ eps*I on the diagonal block)
                            diag = lo <= mb * P < lo + 512
                            nc.tensor.matmul(
                                pb[:, sl], mtile, xt[:, gsl],
                                start=True, stop=not diag,
                            )
                            if diag:
                                off = mb * P - lo + j * 512
                                nc.tensor.matmul(
                                    pb[:, off:off + P], ident_bf[:], epseye[:],
                                    start=False, stop=True,
                                )
                        e = epool.tile([P, HALF], f32)
                        nc.scalar.activation(
                            e[:], pa[:], Act.Exp,
                            bias=sqcol[:, mb:mb + 1], scale=1.0,
                        )
                        o = opool.tile([P, HALF], f32)
                        nc.vector.scalar_tensor_tensor(
                            o[:], pb[:], const_val, e[:], Alu.add, Alu.add
                        )
                        nc.sync.dma_start(
                            out_ap[mb * P:(mb + 1) * P, h * HALF:(h + 1) * HALF],
                            o[:],
                        )

    nc.compile()
    return nc


def _get_nc(c: float, const_val: float):
    key = (c, const_val)
    if key not in _CACHE:
        _CACHE[key] = _build(c, const_val)
    return _CACHE[key]


def kernel(features, const, scale):
    from concourse.bass_utils import run_bass_kernel_spmd

    features = np.ascontiguousarray(features, dtype=np.float32)
    const_val = float(np.asarray(const).reshape(-1)[0])
    scale_arr = np.asarray(scale, dtype=np.float32).reshape(-1)
    assert features.shape == (B, T, C)
    assert scale_arr.shape == (T,)
    if not np.all(scale_arr == scale_arr[0]):
        raise NotImplementedError("non-uniform scale path not implemented yet")
    c = float(1.0 / (2.0 * float(scale_arr[0]) ** 2))

    nc = _get_nc(c, const_val)
    in_maps = [{"x": features[b]} for b in range(B)]
    res = run_bass_kernel_spmd(nc, in_maps, core_ids=list(range(B)))
    return np.stack([res.results[b]["out"] for b in range(B)], axis=0)


# revision 5
# speedup vs baseline: 1.1696x; 1.1696x over previous
"""Trainium2 Bass kernel for nn_KernelBlock_7387343749286 (sparse_attention).

Computes, for features [B=8, T=2048, C=128], const [1], scale [T]:
    gram[b,t,s] = <features[b,t,:], features[b,s,:]>
    K = (gram + const) + exp(-(sq_t + sq_s - 2*gram) / (2*scale_s^2)) + eps*I

Sharding: batch b across the 8 NeuronCores (data parallel), one 2048x2048
output per core. Within a core the T x T Gram matrix is tiled into
[128 x 1024] blocks flash-attention style.

Per-core device algorithm (uniform scale sigma, c = 1/(2*sigma^2)):
  X^T via PE transposes -> xt (f32r) and a bf16 copy xbf.
  bank A (PSUM) = gram_bf16 - (sq_s - C0)/2 via one bf16 matmul plus a
      K=1 rank-1 bf16 accumulate; ScalarE computes
      E = exp(c*A + bias_t), bias_t = -(c/2)(sq_t + C0) per-partition.
      All sq values derive from the SAME bf16-rounded features, so the
      diagonal exp(0)=1 cancellation is preserved.
  bank B (PSUM) = gram in f32r (+ eps*I on diagonal blocks via a bf16
      identity accumulate) for the linear-kernel term's accuracy.
  VectorE fuses the rest in one pass: out = (B + const) + E.
"""

import numpy as np

B, T, C = 8, 2048, 128
EPSILON = 1e-5
P = 128            # partitions
NB = T // P        # 16 row blocks
HALF = 1024        # column tile width (2 PSUM banks)
NH = T // HALF     # 2 column halves
C0 = float(C)      # centering constant for sq values (E[sq] = C)

_CACHE = {}


def _build(c: float, const_val: float):
    import concourse.bass as bass
    import concourse.mybir as mybir
    from concourse import bacc
    from concourse.tile import TileContext
    from concourse.masks import make_identity

    f32 = mybir.dt.float32
    f32r = mybir.dt.float32r
    bf16 = mybir.dt.bfloat16
    Alu = mybir.AluOpType
    Act = mybir.ActivationFunctionType

    nc = bacc.Bacc("TRN2", target_bir_lowering=False, debug=False)
    x = nc.dram_tensor("x", (T, C), f32, kind="ExternalInput")
    out = nc.dram_tensor("out", (T, T), f32, kind="ExternalOutput")
    x_ap = x.ap()
    out_ap = out.ap()

    with TileContext(nc) as tc:
        with (
            tc.tile_pool(name="const_pool", bufs=1) as cpool,
            tc.tile_pool(name="work_pool", bufs=1) as wpool,
        ):
            # ---------------- prologue ----------------
            ident = cpool.tile([P, P], f32)
            make_identity(nc, ident)
            epseye = cpool.tile([P, P], bf16)
            nc.vector.tensor_scalar_mul(epseye[:], ident[:], EPSILON)
            ident_bf = cpool.tile([P, P], bf16)
            nc.vector.tensor_copy(ident_bf[:], ident[:])

            ones_bf = cpool.tile([1, P], bf16)
            nc.vector.memset(ones_bf[:], 1.0)
            neghalf = cpool.tile([P, 1], f32)
            nc.vector.memset(neghalf[:], -0.5)

            # natural-layout X: partition = t within block, free = (block, c)
            xnat = wpool.tile([P, T], f32)
            x_blocked = x_ap.rearrange("(mb p) c -> p mb c", p=P)
            for mb in range(NB):
                nc.sync.dma_start(
                    xnat[:, mb * C:(mb + 1) * C], x_blocked[:, mb, :]
                )

            # bf16-rounded natural X -> per-row sums of squares (ACT, with
            # accum_out) -> sqcol; runs while the PE transposes below.
            xnbf = wpool.tile([P, T], bf16)
            sqcol_raw = cpool.tile([P, NB], f32)
            scratch = wpool.tile([P, P], f32)
            sqcol = cpool.tile([P, NB], f32)

            xt = cpool.tile([P, T], f32r)   # X^T: partition = c, free = t
            xbf = cpool.tile([P, T], bf16)  # bf16(X^T)
            with tc.tile_pool(name="tp_psum", bufs=4, space="PSUM") as tpp:
                for mb in range(NB):
                    sl = slice(mb * P, (mb + 1) * P)
                    pt = tpp.tile([P, P], f32)
                    nc.tensor.transpose(pt[:], xnat[:, sl], ident[:])
                    nc.scalar.copy(xt[:, sl], pt[:])
                    nc.vector.tensor_copy(xbf[:, sl], pt[:])
                    nc.vector.tensor_copy(xnbf[:, sl], xnat[:, sl])
                    nc.scalar.activation(
                        scratch[:], xnbf[:, sl], Act.Square,
                        accum_out=sqcol_raw[:, mb:mb + 1],
                    )

            # ACT bias: -(c/2) * (sq_t + C0)
            nc.vector.tensor_scalar(
                sqcol[:], sqcol_raw[:], -0.5 * c, -0.5 * c * C0, Alu.mult, Alu.add
            )

            # squares of bf16 features for the column sums
            xtsq = wpool.tile([P, T], f32)
            nc.vector.tensor_tensor(xtsq[:], xbf[:], xbf[:], Alu.mult)

            # sqrow2[s] = -(sq_s - C0)/2, bf16 (rank-1 rhs; c applied in ACT)
            sqrow2 = cpool.tile([1, T], bf16)
            with tc.tile_pool(name="sr_psum", bufs=4, space="PSUM") as srp:
                for j in range(T // 512):
                    pr = srp.tile([1, 512], f32)
                    nc.tensor.matmul(
                        pr[:], neghalf[:], xtsq[:, j * 512:(j + 1) * 512],
                        start=True, stop=True,
                    )
                    nc.scalar.activation(
                        sqrow2[0:1, j * 512:(j + 1) * 512], pr[:],
                        Act.Copy, bias=0.5 * C0, scale=1.0,
                    )

            # ---------------- main loop ----------------
            with (
                tc.tile_pool(name="pa_psum", bufs=2, space="PSUM") as pap,
                tc.tile_pool(name="pb_psum", bufs=2, space="PSUM") as pbp,
                tc.tile_pool(name="e_pool", bufs=3) as epool,
                tc.tile_pool(name="o_pool", bufs=3) as opool,
            ):
                for mb in range(NB):
                    mrow = slice(mb * P, (mb + 1) * P)
                    for h in range(NH):
                        pa = pap.tile([P, HALF], f32)
                        pb = pbp.tile([P, HALF], f32)
                        for j in range(HALF // 512):
                            lo = h * HALF + j * 512          # global col offset
                            sl = slice(j * 512, (j + 1) * 512)  # psum cols
                            gsl = slice(lo, lo + 512)        # xt cols
                            # bank A: bf16 gram + rank-1 column term
                            nc.tensor.matmul(
                                pa[:, sl], xbf[:, mrow], xbf[:, gsl],
                                start=True, stop=False,
                            )
                            nc.tensor.matmul(
                                pa[:, sl], ones_bf[:], sqrow2[0:1, gsl],
                                start=False, stop=True,
                            )
                            # bank B: f32r gram (+ eps*I on the diagonal block)
                            diag = lo <= mb * P < lo + 512
                            nc.tensor.matmul(
                                pb[:, sl], xt[:, mrow], xt[:, gsl],
                                start=True, stop=not diag,
                            )
                            if diag:
                                off = mb * P - lo + j * 512
                                nc.tensor.matmul(
                                    pb[:, off:off + P], ident_bf[:], epseye[:],
                                    start=False, stop=True,
                                )
                        e = epool.tile([P, HALF], f32)
                        nc.scalar.activation(
                            e[:], pa[:], Act.Exp,
                            bias=sqcol[:, mb:mb + 1], scale=c,
                        )
                        o = opool.tile([P, HALF], f32)
                        nc.vector.scalar_tensor_tensor(
                            o[:], pb[:], const_val, e[:], Alu.add, Alu.add
                        )
                        nc.sync.dma_start(
                            out_ap[mrow, h * HALF:(h + 1) * HALF], o[:]
                        )

    nc.compile()
    return nc


def _get_nc(c: float, const_val: float):
    key = (c, const_val)
    if key not in _CACHE:
        _CACHE[key] = _build(c, const_val)
    return _CACHE[key]


def kernel(features, const, scale):
    from concourse.bass_utils import run_bass_kernel_spmd

    features = np.ascontiguousarray(features, dtype=np.float32)
    const_val = float(np.asarray(const).reshape(-1)[0])
    scale_arr = np.asarray(scale, dtype=np.float32).reshape(-1)
    assert features.shape == (B, T, C)
    assert scale_arr.shape == (T,)
    if not np.all(scale_arr == scale_arr[0]):
        raise NotImplementedError("non-uniform scale path not implemented yet")
    c = float(1.0 / (2.0 * float(scale_arr[0]) ** 2))

    nc = _get_nc(c, const_val)
    in_maps = [{"x": features[b]} for b in range(B)]
    res = run_bass_kernel_spmd(nc, in_maps, core_ids=list(range(B)))
    return np.stack([res.results[b]["out"] for b in range(B)], axis=0)
